# revision 15
# baseline (speedup 1.0000x reference)
"""ActTransNet Trainium2 kernel (8 NeuronCores, SPMD).

Computation (per reference):
    p_avg = mean_t(precondition)            [B, 2048]
    e_avg = mean_t(effect)                  [B, 2048]
    p_embed = p_avg @ Wp.T + bp             [B, 256]
    e_embed = e_avg @ We.T + be             [B, 256]   <- output 1
    p_trans = W_trans[action] @ p_embed     [B, 256]   <- output 0

Strategy:
  - Batch is argsorted by action on the host; core i gets sorted samples
    [512*i, 512*(i+1)).  Sorting makes each core's distinct-action count
    ~B_unique/8 (~125), so each needed [256,256] table row is DMA'd once.
  - Effect path is sharded in original order (no routing needed).
  - Mean-pool over T=16 runs on the TensorEngine via a block-diagonal
    ones matrix (8 samples/tile, full 128-partition contraction); the
    1/16 is folded into the host-transposed projection weights.
  - Projection: PE matmul contracting over 2048 (pooled activations are
    PE-transposed to put the contraction dim on partitions).
  - MoE stage: host packs each per-core run of equal-action samples into
    "slots" of <=16 samples; 8 slots form a group of 128.  Per group the
    device DMAs the 8 (pre-transposed, pre-swizzled) W matrices, does an
    indirect row-gather of the needed p_embed rows, a PE transpose, 16
    small matmuls, and an indirect row-scatter into the output.  Padded
    lanes carry out-of-bounds indices and are dropped by the DMA bounds
    check.
"""

import numpy as np

B, T, C, D, NA = 4096, 16, 2048, 256, 1000
M = 8            # cores
BS = B // M      # 512 samples per core
NB = BS // 128   # 4 blocks of 128 samples
S_MAX = 32       # samples per slot (PE out base partition must be 0/32/64/96)
PAD = 1 << 20    # out-of-bounds index -> dropped by bounds check

_CACHE = {}

# Profiling hooks for test/dev (harness just calls kernel()).
TRACE = False
TRACE_KWARGS = {}
LAST_RES = None


def _build(nslots):
    import concourse.bacc as bacc
    import concourse.mybir as mybir
    import concourse.tile as tile
    from concourse import bass

    f32 = mybir.dt.float32
    i32 = mybir.dt.int32
    G = nslots // 4

    nc = bacc.Bacc()
    prec = nc.declare_dram_parameter("prec", [BS * T, C], f32, isOutput=False)
    eff = nc.declare_dram_parameter("eff", [BS * T, C], f32, isOutput=False)
    wpt = nc.declare_dram_parameter("wpt", [C, D], f32, isOutput=False)
    wet = nc.declare_dram_parameter("wet", [C, D], f32, isOutput=False)
    bpp = nc.declare_dram_parameter("bp", [1, D], f32, isOutput=False)
    bee = nc.declare_dram_parameter("be", [1, D], f32, isOutput=False)
    ones16 = nc.declare_dram_parameter("ones16", [128, 8], f32, isOutput=False)
    ones1 = nc.declare_dram_parameter("ones1", [1, 128], f32, isOutput=False)
    ident = nc.declare_dram_parameter("ident", [128, 128], f32, isOutput=False)
    wg = nc.declare_dram_parameter("wg", [G, 128, 2048], f32, isOutput=False)
    gidx = nc.declare_dram_parameter("gidx", [G, 128, 1], i32, isOutput=False)
    out_p = nc.declare_dram_parameter("out_p", [BS, D], f32, isOutput=True)
    out_e = nc.declare_dram_parameter("out_e", [BS, D], f32, isOutput=True)

    with tile.TileContext(nc) as tc:
        with (
            tc.tile_pool(name="const", bufs=1) as cpool,
            tc.tile_pool(name="instream", bufs=4) as in_pool,
            tc.tile_pool(name="pavgT", bufs=2) as pavgT_pool,
            tc.tile_pool(name="emb", bufs=2) as emb_pool,
            tc.tile_pool(name="wgp", bufs=2) as wg_pool,
            tc.tile_pool(name="psel", bufs=2) as psel_pool,
            tc.tile_pool(name="pselT", bufs=2) as pselT_pool,
            tc.tile_pool(name="cout", bufs=2) as cout_pool,
            tc.tile_pool(name="idx", bufs=2) as idx_pool,
            tc.tile_pool(name="pp", bufs=1, space="PSUM") as pool_psum,
            tc.tile_pool(name="tp", bufs=2, space="PSUM") as t_psum,
            tc.tile_pool(name="pj", bufs=1, space="PSUM") as proj_psum,
            tc.tile_pool(name="cp", bufs=1, space="PSUM") as c_psum,
            tc.tile_pool(name="dram", bufs=1, space="DRAM") as dram_pool,
        ):
            wpt_sb = cpool.tile([128, 16, D], f32, tag="wpt")
            nc.sync.dma_start(
                wpt_sb[:], wpt[:].rearrange("(c p) i -> p c i", p=128)
            )
            wet_sb = cpool.tile([128, 16, D], f32, tag="wet")
            nc.sync.dma_start(
                wet_sb[:], wet[:].rearrange("(c p) i -> p c i", p=128)
            )
            bp_sb = cpool.tile([1, D], f32, tag="bp")
            nc.sync.dma_start(bp_sb[:], bpp[:])
            be_sb = cpool.tile([1, D], f32, tag="be")
            nc.sync.dma_start(be_sb[:], bee[:])
            ones16_sb = cpool.tile([128, 8], f32, tag="ones16")
            nc.sync.dma_start(ones16_sb[:], ones16[:])
            ones1_sb = cpool.tile([1, 128], f32, tag="ones1")
            nc.sync.dma_start(ones1_sb[:], ones1[:])
            ident_sb = cpool.tile([128, 128], f32, tag="ident")
            nc.sync.dma_start(ident_sb[:], ident[:])

            pe_dram = dram_pool.tile([BS, D], f32)

            # ---- pool + project (both tensors) ----
            # Pooling matmul uses the input chunk as the stationary (lhsT)
            # operand, so the pooled sums come out already transposed:
            # psj[c, b] = sum_t in[(b,t), c].
            for src, wt_sb, b_sb, is_p in ((prec, wpt_sb, bp_sb, True),
                                           (eff, wet_sb, be_sb, False)):
                for blk in range(NB):
                    pavgT = pavgT_pool.tile([128, 16, 128], f32)
                    for j in range(16):
                        it = in_pool.tile([128, C], f32, tag="instream")
                        r0 = (blk * 16 + j) * 128
                        nc.sync.dma_start(it[:], src[r0:r0 + 128, :])
                        psj = pool_psum.tile([128, 16, 8], f32)
                        for c in range(16):
                            nc.tensor.matmul(
                                psj[:, c, :],
                                lhsT=it[:, 128 * c:128 * (c + 1)],
                                rhs=ones16_sb[:],
                                start=True, stop=True,
                            )
                        nc.vector.tensor_copy(
                            pavgT[:, :, 8 * j:8 * (j + 1)], psj[:]
                        )
                    pj = proj_psum.tile([128, D], f32)
                    for c in range(16):
                        nc.tensor.matmul(
                            pj[:],
                            lhsT=pavgT[:, c, :],
                            rhs=wt_sb[:, c, :],
                            start=(c == 0), stop=False,
                        )
                    nc.tensor.matmul(
                        pj[:], lhsT=ones1_sb[:], rhs=b_sb[:],
                        start=False, stop=True,
                    )
                    emb = emb_pool.tile([128, D], f32)
                    nc.vector.tensor_copy(emb[:], pj[:])
                    r0 = blk * 128
                    if is_p:
                        nc.sync.dma_start(pe_dram[r0:r0 + 128, :], emb[:])
                    else:
                        nc.sync.dma_start(out_e[r0:r0 + 128, :], emb[:])

            # ---- MoE transform ----
            for g in range(G):
                wg_sb = wg_pool.tile([128, 2048], f32)
                nc.sync.dma_start(wg_sb[:], wg[g, :, :])
                ix = idx_pool.tile([128, 1], i32)
                nc.sync.dma_start(ix[:], gidx[g, :, :])
                psel = psel_pool.tile([128, D], f32)
                nc.gpsimd.indirect_dma_start(
                    out=psel[:], out_offset=None,
                    in_=pe_dram[:],
                    in_offset=bass.IndirectOffsetOnAxis(ap=ix[:, :1], axis=0),
                    bounds_check=BS - 1, oob_is_err=False,
                )
                pselT = pselT_pool.tile([128, D], f32)
                for c in range(2):
                    tp = t_psum.tile([128, 128], f32, tag="tp")
                    nc.tensor.transpose(
                        tp[:], psel[:, 128 * c:128 * (c + 1)], ident_sb[:]
                    )
                    nc.vector.tensor_copy(
                        pselT[:, 128 * c:128 * (c + 1)], tp[:]
                    )
                cps = c_psum.tile([128, D], f32)
                for b4 in range(4):
                    for c in range(2):
                        nc.tensor.matmul(
                            cps[32 * b4:32 * (b4 + 1), :],
                            lhsT=pselT[:, 128 * c + 32 * b4:128 * c + 32 * (b4 + 1)],
                            rhs=wg_sb[:, (2 * b4 + c) * D:(2 * b4 + c + 1) * D],
                            start=(c == 0), stop=(c == 1),
                            tile_position=(0, 32 * b4),
                        )
                co = cout_pool.tile([128, D], f32)
                nc.vector.tensor_copy(co[:], cps[:])
                nc.gpsimd.indirect_dma_start(
                    out=out_p[:],
                    out_offset=bass.IndirectOffsetOnAxis(ap=ix[:, :1], axis=0),
                    in_=co[:], in_offset=None,
                    bounds_check=BS - 1, oob_is_err=False,
                )

    nc.compile()
    return nc


def _get_program(nslots):
    if nslots not in _CACHE:
        _CACHE[nslots] = _build(nslots)
    return _CACHE[nslots]


def kernel(precondition, effect, action, Wp, bp, We, be, W_trans):
    from concourse.bass_utils import run_bass_kernel_spmd

    precondition = np.asarray(precondition, dtype=np.float32)
    effect = np.asarray(effect, dtype=np.float32)
    act = np.asarray(action).astype(np.int64)
    Wp = np.asarray(Wp, dtype=np.float32)
    bp = np.asarray(bp, dtype=np.float32)
    We = np.asarray(We, dtype=np.float32)
    be = np.asarray(be, dtype=np.float32)
    W_trans = np.asarray(W_trans, dtype=np.float32)

    order = np.argsort(act, kind="stable")
    a_s = act[order]

    # Pack each core's sorted samples into slots of <=S_MAX equal-action rows.
    per_core = []
    max_slots = 0
    for i in range(M):
        ai = a_s[i * BS:(i + 1) * BS]
        bounds = np.flatnonzero(np.diff(ai)) + 1
        starts = np.concatenate([[0], bounds])
        ends = np.concatenate([bounds, [BS]])
        sl = []
        for s0, s1 in zip(starts, ends):
            a = int(ai[s0])
            for cs in range(int(s0), int(s1), S_MAX):
                sl.append((a, cs, min(cs + S_MAX, int(s1))))
        per_core.append(sl)
        max_slots = max(max_slots, len(sl))

    nslots = max(160, ((max_slots + 31) // 32) * 32)
    nc = _get_program(nslots)
    G = nslots // 4

    WpT = np.ascontiguousarray(Wp.T) / np.float32(T)
    WeT = np.ascontiguousarray(We.T) / np.float32(T)
    ones16 = np.zeros([128, 8], np.float32)
    ones16[np.arange(128), np.arange(128) // 16] = 1.0
    ones1 = np.ones([1, 128], np.float32)
    ident = np.eye(128, dtype=np.float32)
    bp2 = bp.reshape(1, D)
    be2 = be.reshape(1, D)

    in_maps = []
    for i in range(M):
        rows = order[i * BS:(i + 1) * BS]
        prec_i = np.ascontiguousarray(
            precondition[rows].reshape(BS * T, C))
        eff_i = np.ascontiguousarray(
            effect[i * BS:(i + 1) * BS].reshape(BS * T, C))
        sl = per_core[i]
        acts = np.zeros([nslots], np.int64)
        gidx = np.full([nslots, S_MAX], PAD, np.int32)
        for r, (a, s0, s1) in enumerate(sl):
            acts[r] = a
            gidx[r, :s1 - s0] = np.arange(s0, s1, dtype=np.int32)
        # wg[g, p, (r4*2+c)*256 + i] = W_trans[acts[4g+r4], i, 128c+p]
        wgf = W_trans[acts]  # [nslots, i, j]
        wgr = wgf.reshape(G, 4, D, 2, 128)          # (g, r4, i, c, p)
        wg = np.ascontiguousarray(
            wgr.transpose(0, 4, 1, 3, 2)).reshape(G, 128, 2048)
        in_maps.append({
            "prec": prec_i, "eff": eff_i,
            "wpt": WpT, "wet": WeT, "bp": bp2, "be": be2,
            "ones16": ones16, "ones1": ones1, "ident": ident,
            "wg": wg, "gidx": np.ascontiguousarray(gidx.reshape(G, 128, 1)),
        })

    global LAST_RES
    res = run_bass_kernel_spmd(nc, in_maps, list(range(M)),
                               trace=TRACE, **TRACE_KWARGS)
    LAST_RES = res

    out_p = np.empty([B, D], np.float32)
    out_e = np.empty([B, D], np.float32)
    for i in range(M):
        out_p[order[i * BS:(i + 1) * BS]] = res.results[i]["out_p"]
        out_e[i * BS:(i + 1) * BS] = res.results[i]["out_e"]
    return out_p, out_e


# revision 27
# speedup vs baseline: 58.9740x; 58.9740x over previous
"""ActTransNet Trainium2 kernel (8 NeuronCores, SPMD).

Computation (per reference):
    p_avg = mean_t(precondition)            [B, 2048]
    e_avg = mean_t(effect)                  [B, 2048]
    p_embed = p_avg @ Wp.T + bp             [B, 256]
    e_embed = e_avg @ We.T + be             [B, 256]   <- output 1
    p_trans = W_trans[action] @ p_embed     [B, 256]   <- output 0

Strategy:
  - Batch is argsorted by action on the host; core i gets sorted samples
    [512*i, 512*(i+1)).  Sorting makes each core's distinct-action count
    ~B_unique/8 (~125), so each needed [256,256] table row is DMA'd once.
  - Effect path is sharded in original order (no routing needed).
  - Mean-pool over T=16 runs on the TensorEngine via a block-diagonal
    ones matrix (8 samples/tile, full 128-partition contraction); the
    1/16 is folded into the host-transposed projection weights.
  - Projection: PE matmul contracting over 2048 (pooled activations are
    PE-transposed to put the contraction dim on partitions).
  - MoE stage: host packs each per-core run of equal-action samples into
    "slots" of <=16 samples; 8 slots form a group of 128.  Per group the
    device DMAs the 8 (pre-transposed, pre-swizzled) W matrices, does an
    indirect row-gather of the needed p_embed rows, a PE transpose, 16
    small matmuls, and an indirect row-scatter into the output.  Padded
    lanes carry out-of-bounds indices and are dropped by the DMA bounds
    check.
"""

import numpy as np

B, T, C, D, NA = 4096, 16, 2048, 256, 1000
M = 8            # cores
BS = B // M      # 512 samples per core
NB = BS // 128   # 4 blocks of 128 samples
S_MAX = 32       # samples per slot (PE out base partition must be 0/32/64/96)
PAD = 1 << 20    # out-of-bounds index -> dropped by bounds check

_CACHE = {}

# Profiling hooks for test/dev (harness just calls kernel()).
TRACE = False
TRACE_KWARGS = {}
LAST_RES = None


def _build(nslots):
    import concourse.bacc as bacc
    import concourse.mybir as mybir
    import concourse.tile as tile
    from concourse import bass

    f32 = mybir.dt.float32
    i32 = mybir.dt.int32
    G = nslots // 4

    nc = bacc.Bacc()
    prec = nc.declare_dram_parameter("prec", [BS * T, C], f32, isOutput=False)
    eff = nc.declare_dram_parameter("eff", [BS * T, C], f32, isOutput=False)
    wpt = nc.declare_dram_parameter("wpt", [C, D], f32, isOutput=False)
    wet = nc.declare_dram_parameter("wet", [C, D], f32, isOutput=False)
    bpp = nc.declare_dram_parameter("bp", [1, D], f32, isOutput=False)
    bee = nc.declare_dram_parameter("be", [1, D], f32, isOutput=False)
    ones16 = nc.declare_dram_parameter("ones16", [128, 8], f32, isOutput=False)
    ones1 = nc.declare_dram_parameter("ones1", [1, 128], f32, isOutput=False)
    ident = nc.declare_dram_parameter("ident", [128, 128], f32, isOutput=False)
    wg = nc.declare_dram_parameter("wg", [G, 128, 2048], f32, isOutput=False)
    gidx = nc.declare_dram_parameter("gidx", [G, 128, 1], i32, isOutput=False)
    out_p = nc.declare_dram_parameter("out_p", [BS, D], f32, isOutput=True)
    out_e = nc.declare_dram_parameter("out_e", [BS, D], f32, isOutput=True)

    with tile.TileContext(nc) as tc:
        with (
            tc.tile_pool(name="const", bufs=1) as cpool,
            tc.tile_pool(name="instream", bufs=8) as in_pool,
            tc.tile_pool(name="pavgT", bufs=2) as pavgT_pool,
            tc.tile_pool(name="emb", bufs=2) as emb_pool,
            tc.tile_pool(name="wgp", bufs=4) as wg_pool,
            tc.tile_pool(name="psel", bufs=3) as psel_pool,
            tc.tile_pool(name="pselT", bufs=3) as pselT_pool,
            tc.tile_pool(name="cout", bufs=3) as cout_pool,
            tc.tile_pool(name="idx", bufs=4) as idx_pool,
            tc.tile_pool(name="pp", bufs=2, space="PSUM") as pool_psum,
            tc.tile_pool(name="tp", bufs=2, space="PSUM") as t_psum,
            tc.tile_pool(name="pj", bufs=2, space="PSUM") as proj_psum,
            tc.tile_pool(name="cp", bufs=2, space="PSUM") as c_psum,
            tc.tile_pool(name="dram", bufs=1, space="DRAM") as dram_pool,
        ):
            wpt_sb = cpool.tile([128, 16, D], f32, tag="wpt")
            nc.sync.dma_start(
                wpt_sb[:], wpt[:].rearrange("(c p) i -> p c i", p=128)
            )
            wet_sb = cpool.tile([128, 16, D], f32, tag="wet")
            nc.sync.dma_start(
                wet_sb[:], wet[:].rearrange("(c p) i -> p c i", p=128)
            )
            bp_sb = cpool.tile([1, D], f32, tag="bp")
            nc.sync.dma_start(bp_sb[:], bpp[:])
            be_sb = cpool.tile([1, D], f32, tag="be")
            nc.sync.dma_start(be_sb[:], bee[:])
            ones16_sb = cpool.tile([128, 8], f32, tag="ones16")
            nc.sync.dma_start(ones16_sb[:], ones16[:])
            ones1_sb = cpool.tile([1, 128], f32, tag="ones1")
            nc.sync.dma_start(ones1_sb[:], ones1[:])
            ident_sb = cpool.tile([128, 128], f32, tag="ident")
            nc.sync.dma_start(ident_sb[:], ident[:])

            pe_dram = dram_pool.tile([BS, D], f32)

            # ---- pool + project ----
            # Pooling matmul uses the input chunk as the stationary (lhsT)
            # operand, so the pooled sums come out already transposed:
            # psj[c, b] = sum_t in[(b,t), c].
            def stage_a_steps(src, wt_sb, b_sb, dst):
                for blk in range(NB):
                    pavgT = pavgT_pool.tile([128, 16, 128], f32)
                    for j in range(16):
                        it = in_pool.tile([128, C], f32, tag="instream")
                        r0 = (blk * 16 + j) * 128
                        nc.sync.dma_start(it[:], src[r0:r0 + 128, :])
                        psj = pool_psum.tile([128, 16, 8], f32)
                        for c in range(16):
                            nc.tensor.matmul(
                                psj[:, c, :],
                                lhsT=it[:, 128 * c:128 * (c + 1)],
                                rhs=ones16_sb[:],
                                start=True, stop=True,
                            )
                        nc.vector.tensor_copy(
                            pavgT[:, :, 8 * j:8 * (j + 1)], psj[:]
                        )
                        if j < 15:
                            yield
                    pj = proj_psum.tile([128, D], f32)
                    for c in range(16):
                        nc.tensor.matmul(
                            pj[:],
                            lhsT=pavgT[:, c, :],
                            rhs=wt_sb[:, c, :],
                            start=(c == 0), stop=False,
                        )
                    nc.tensor.matmul(
                        pj[:], lhsT=ones1_sb[:], rhs=b_sb[:],
                        start=False, stop=True,
                    )
                    emb = emb_pool.tile([128, D], f32)
                    nc.vector.tensor_copy(emb[:], pj[:])
                    r0 = blk * 128
                    nc.sync.dma_start(dst[r0:r0 + 128, :], emb[:])
                    yield

            def stage_c_steps():
                for g in range(G):
                    wg_sb = wg_pool.tile([128, 2048], f32)
                    nc.gpsimd.dma_start(wg_sb[:], wg[g, :, :])
                    ix = idx_pool.tile([128, 1], i32)
                    nc.gpsimd.dma_start(ix[:], gidx[g, :, :])
                    psel = psel_pool.tile([128, D], f32)
                    nc.gpsimd.indirect_dma_start(
                        out=psel[:], out_offset=None,
                        in_=pe_dram[:],
                        in_offset=bass.IndirectOffsetOnAxis(ap=ix[:, :1], axis=0),
                        bounds_check=BS - 1, oob_is_err=False,
                    )
                    pselT = pselT_pool.tile([128, D], f32)
                    for c in range(2):
                        tp = t_psum.tile([128, 128], f32, tag="tp")
                        nc.tensor.transpose(
                            tp[:], psel[:, 128 * c:128 * (c + 1)], ident_sb[:]
                        )
                        nc.vector.tensor_copy(
                            pselT[:, 128 * c:128 * (c + 1)], tp[:]
                        )
                    cps = c_psum.tile([128, D], f32)
                    for b4 in range(4):
                        for c in range(2):
                            nc.tensor.matmul(
                                cps[32 * b4:32 * (b4 + 1), :],
                                lhsT=pselT[:, 128 * c + 32 * b4:128 * c + 32 * (b4 + 1)],
                                rhs=wg_sb[:, (2 * b4 + c) * D:(2 * b4 + c + 1) * D],
                                start=(c == 0), stop=(c == 1),
                                tile_position=(0, 32 * b4),
                            )
                    co = cout_pool.tile([128, D], f32)
                    nc.vector.tensor_copy(co[:], cps[:])
                    nc.gpsimd.indirect_dma_start(
                        out=out_p[:],
                        out_offset=bass.IndirectOffsetOnAxis(ap=ix[:, :1], axis=0),
                        in_=co[:], in_offset=None,
                        bounds_check=BS - 1, oob_is_err=False,
                    )
                    yield

            # precondition stream first (stage C needs its embeddings),
            # then the MoE groups (self-paced on the gpsimd queue), then
            # the effect stream on the sync queue.
            for _ in stage_a_steps(prec, wpt_sb, bp_sb, pe_dram):
                pass
            for _ in stage_c_steps():
                pass
            for _ in stage_a_steps(eff, wet_sb, be_sb, out_e):
                pass


    nc.compile()
    return nc


def _get_program(nslots):
    if nslots not in _CACHE:
        _CACHE[nslots] = _build(nslots)
    return _CACHE[nslots]


def kernel(precondition, effect, action, Wp, bp, We, be, W_trans):
    from concourse.bass_utils import run_bass_kernel_spmd

    precondition = np.asarray(precondition, dtype=np.float32)
    effect = np.asarray(effect, dtype=np.float32)
    act = np.asarray(action).astype(np.int64)
    Wp = np.asarray(Wp, dtype=np.float32)
    bp = np.asarray(bp, dtype=np.float32)
    We = np.asarray(We, dtype=np.float32)
    be = np.asarray(be, dtype=np.float32)
    W_trans = np.asarray(W_trans, dtype=np.float32)

    order = np.argsort(act, kind="stable")
    a_s = act[order]

    # Pack each core's sorted samples into slots of <=S_MAX equal-action rows.
    per_core = []
    max_slots = 0
    for i in range(M):
        ai = a_s[i * BS:(i + 1) * BS]
        bounds = np.flatnonzero(np.diff(ai)) + 1
        starts = np.concatenate([[0], bounds])
        ends = np.concatenate([bounds, [BS]])
        sl = []
        for s0, s1 in zip(starts, ends):
            a = int(ai[s0])
            for cs in range(int(s0), int(s1), S_MAX):
                sl.append((a, cs, min(cs + S_MAX, int(s1))))
        per_core.append(sl)
        max_slots = max(max_slots, len(sl))

    nslots = max(8, ((max_slots + 7) // 8) * 8)
    nc = _get_program(nslots)
    G = nslots // 4

    WpT = np.ascontiguousarray(Wp.T) / np.float32(T)
    WeT = np.ascontiguousarray(We.T) / np.float32(T)
    ones16 = np.zeros([128, 8], np.float32)
    ones16[np.arange(128), np.arange(128) // 16] = 1.0
    ones1 = np.ones([1, 128], np.float32)
    ident = np.eye(128, dtype=np.float32)
    bp2 = bp.reshape(1, D)
    be2 = be.reshape(1, D)

    in_maps = []
    for i in range(M):
        rows = order[i * BS:(i + 1) * BS]
        prec_i = np.ascontiguousarray(
            precondition[rows].reshape(BS * T, C))
        eff_i = np.ascontiguousarray(
            effect[i * BS:(i + 1) * BS].reshape(BS * T, C))
        sl = per_core[i]
        acts = np.zeros([nslots], np.int64)
        gidx = np.full([nslots, S_MAX], PAD, np.int32)
        for r, (a, s0, s1) in enumerate(sl):
            acts[r] = a
            gidx[r, :s1 - s0] = np.arange(s0, s1, dtype=np.int32)
        # wg[g, p, (r4*2+c)*256 + i] = W_trans[acts[4g+r4], i, 128c+p]
        wgf = W_trans[acts]  # [nslots, i, j]
        wgr = wgf.reshape(G, 4, D, 2, 128)          # (g, r4, i, c, p)
        wg = np.ascontiguousarray(
            wgr.transpose(0, 4, 1, 3, 2)).reshape(G, 128, 2048)
        in_maps.append({
            "prec": prec_i, "eff": eff_i,
            "wpt": WpT, "wet": WeT, "bp": bp2, "be": be2,
            "ones16": ones16, "ones1": ones1, "ident": ident,
            "wg": wg, "gidx": np.ascontiguousarray(gidx.reshape(G, 128, 1)),
        })

    global LAST_RES
    res = run_bass_kernel_spmd(nc, in_maps, list(range(M)),
                               trace=TRACE, **TRACE_KWARGS)
    LAST_RES = res

    out_p = np.empty([B, D], np.float32)
    out_e = np.empty([B, D], np.float32)
    for i in range(M):
        out_p[order[i * BS:(i + 1) * BS]] = res.results[i]["out_p"]
        out_e[i * BS:(i + 1) * BS] = res.results[i]["out_e"]
    return out_p, out_e
